# revision 1
# baseline (speedup 1.0000x reference)
"""Trainium2 Bass kernel for DecoderMultiHeadAttention (B=2, T=2048, C=768, H=12).

Sharding: 8 cores = 2 batches x 4 head-groups (3 heads each).
Per core: compute K,V projections for its head group, RoPE, causal
flash-style attention with transposed score layout, AllGather of
attention outputs within each batch group of 4 cores, then a
column-sharded output projection.

Note: the reference uses q = rope(v) (faithful source bug), so the
q-chunk of W_att (columns C..2C) is never used and is not computed.
"""

import sys

_REPO = "/opt/trn_rl_repo"
if _REPO not in sys.path:
    sys.path.insert(0, _REPO)

import numpy as np

import concourse.bass as bass
import concourse.mybir as mybir
import concourse.tile as tile
from concourse import bacc
from concourse.bass_utils import run_bass_kernel_spmd
from concourse.masks import make_identity

B, T, C, H = 2, 2048, 768, 12
D = C // H            # 64
N_CORES = 8
G = 4                 # head groups
HG = H // G           # 3 heads per group
CG = HG * D           # 192 output columns per group
NT = T // 128         # 16 t-chunks
NCC = C // 128        # 6 c-chunks
TQ = 512              # q block width
F32 = mybir.dt.float32
F32R = mybir.dt.float32r
F16 = mybir.dt.float16
EXP = mybir.ActivationFunctionType.Exp
SCALE = float(D) ** -0.5
SENT = 1024.0         # causal-mask sentinel: exp(scale*0 - scale*SENT) ~ 0


def _body(nc, tc, x, wkv, wp, bp, cos3, sin3, out_t, sim_variant=False, reps=1):
    with tc.tile_pool(name="const", bufs=1) as cp:
        ident = cp.tile([128, 128], F32)
        make_identity(nc, ident[:])
        identR = cp.tile([128, 128], F32R)
        nc.scalar.copy(identR[:], ident[:])
        # tri[p, f] = 1.0 if f >= p else 0.0  (keep tq >= tk in diagonal blocks)
        tri = cp.tile([128, 128], F32)
        nc.gpsimd.memset(tri[:], 1.0)
        nc.gpsimd.affine_select(
            out=tri[:], in_=tri[:], compare_op=mybir.AluOpType.is_ge,
            fill=0.0, base=0, pattern=[[1, 128]], channel_multiplier=-1)

        # cdup/sdup: [128, NT*192], cols i*192 + h*64 + {2j, 2j+1} = cos/sin_j
        # (DMAs issued inside the stage-1 loop so x chunk 0 loads first)
        cos_sb = cp.tile([128, NT * 192], F32)
        sin_sb = cp.tile([128, NT * 192], F32)
        # weights feed f32r matmuls: DRAM-side bitcast (PE rounds internally)
        wkv_sb = cp.tile([128, NCC * 2 * CG], F32R)
        nc.scalar.dma_start(
            wkv_sb[:].rearrange("p (n m) -> p n m", n=NCC),
            wkv.rearrange("(n p) m -> p n m", p=128).bitcast(F32R))
        wp_sb = cp.tile([128, NCC * CG], F32R)
        bp_sb = cp.tile([1, CG], F32R)
        ones_f = cp.tile([1, TQ], F32)
        nc.gpsimd.memset(ones_f[:], 1.0)
        ones_sb = cp.tile([1, TQ], F32R)
        nc.scalar.copy(ones_sb[:], ones_f[:])
        triR = cp.tile([128, 128], F32R)
        nc.scalar.copy(triR[:], tri[:])

        # persistent per-head [D, T] tensors: heads 0,1 packed in one tile
        kT01 = cp.tile([128, T], F32R)
        kT2 = cp.tile([64, T], F32R)
        qT01 = cp.tile([128, T], F32R)
        qT2 = cp.tile([64, T], F32R)
        # V in [T, D] layout with a ones column appended per head: per
        # t-chunk i, cols [i*195 + h*65 : .. + 64] = V_h, col .. + 64 = 1.0
        vaug = cp.tile([128, NT * (HG * 65)], F32R)
        ones48 = cp.tile([128, NT * HG], F32)
        nc.gpsimd.memset(ones48[:], 1.0)
        nc.scalar.copy(
            vaug[:].rearrange("p (k c) -> p k c", c=65)[:, :, 64], ones48[:])
        # attention output, transposed [CG, T], one tile pair per T-quarter
        # so each AllGather depends only on its quarter's writes
        SPLITS = [(0, 512), (512, 512), (1024, 512),
                  (1536, 256), (1792, 256)]
        NSP = len(SPLITS)
        oT01h = [cp.tile([128, bw], F16, name=f"oT01h{k}")
                 for k, (bs_, bw) in enumerate(SPLITS)]
        oT2h = [cp.tile([64, bw], F16, name=f"oT2h{k}")
                for k, (bs_, bw) in enumerate(SPLITS)]

        # stages 1..3, optionally repeated for steady-state timing
        for _rep in range(reps):
            # ---- Stage 1: KV projection + RoPE + transposes ----
            # SBUF pools for all stages stay open together (no release
            # barriers); only the PSUM pool swaps between stage 1 and 2+3
            with tc.tile_pool(name="s1", bufs=3) as s1, \
                 tc.tile_pool(name="s2", bufs=2) as s2, \
                 tc.tile_pool(name="s3", bufs=1) as s3:
              with tc.tile_pool(name="s1ps", bufs=2, space="PSUM") as s1ps:
                for i in range(NT):
                    x_sb = s1.tile([128, C], F32R, tag="x", bufs=4)
                    nc.sync.dma_start(x_sb[:],
                                      x[i * 128:(i + 1) * 128, :].bitcast(F32R))
                    if i in (0, 2, 4, 6):
                        # rope tables in quarters on the scalar HWDGE queue,
                        # in parallel with x chunk loads on the sync queue
                        qq = slice((i // 2) * (NT // 4) * 192,
                                   ((i // 2) + 1) * (NT // 4) * 192)
                        nc.scalar.dma_start(cos_sb[:, qq], cos3[:, qq])
                        nc.scalar.dma_start(sin_sb[:, qq], sin3[:, qq])
                    # batched PE transpose of the whole [128, C] chunk
                    xtp = s1ps.tile([128, C], F32R, tag="xtp", bufs=2)
                    for c in range(NCC):
                        nc.tensor.transpose(xtp[:, c * 128:(c + 1) * 128],
                                            x_sb[:, c * 128:(c + 1) * 128],
                                            identR[:])
                    xT_sb = s1.tile([128, C], F32R, tag="xTs")
                    nc.scalar.copy(xT_sb[:], xtp[:])
                    kv_ps = s1ps.tile([128, 2 * CG], F32, tag="kv")
                    for c in range(NCC):
                        nc.tensor.matmul(
                            kv_ps[:], xT_sb[:, c * 128:(c + 1) * 128],
                            wkv_sb[:, c * 2 * CG:(c + 1) * 2 * CG],
                            start=(c == 0), stop=(c == NCC - 1))
                    # SBUF staging of K|V (ACT): rope reads then hit the DVE
                    # fp32 SBUF fast path, and vaug copies from SBUF too
                    kv_sb = s1.tile([128, 2 * CG], F32, tag="kvs")
                    nc.scalar.copy(kv_sb[:], kv_ps[:])

                    # RoPE (4 DVE ops per half): K half -> kq[:, 0:CG],
                    # Q = rope(V) half -> kq[:, CG:2CG]
                    kq_sb = s1.tile([128, 2 * CG], F32R, tag="kq")
                    cS = cos_sb[:, i * CG:(i + 1) * CG]
                    sS = sin_sb[:, i * CG:(i + 1) * CG]
                    for off in (0, CG):
                        kvh = kv_sb[:, off:off + CG]
                        a_sb = s1.tile([128, CG], F32, tag="ra")
                        b_sb = s1.tile([128, CG], F32, tag="rb")
                        nc.vector.tensor_mul(a_sb[:], kvh, cS)
                        nc.gpsimd.tensor_mul(b_sb[:], kvh, sS)
                        nc.vector.tensor_sub(kq_sb[:, off:off + CG:2],
                                             a_sb[:, 0:CG:2], b_sb[:, 1:CG:2])
                        nc.vector.tensor_add(kq_sb[:, off + 1:off + CG:2],
                                             b_sb[:, 0:CG:2], a_sb[:, 1:CG:2])

                    # V (unroped) into vaug [T, 65*3] layout
                    vdst = vaug[:, i * 195:(i + 1) * 195] \
                        .rearrange("p (h c) -> p h c", h=HG)[:, :, 0:64]
                    vsrc = kv_sb[:, CG:2 * CG].rearrange("p (h c) -> p h c", h=HG)
                    nc.vector.tensor_copy(vdst, vsrc)

                    # transpose roped K and Q into [D, T] per-head layouts;
                    # all 4 transposes share one single-bank PSUM tile
                    tp = s1ps.tile([128, 4 * 128], F32R, tag="tp")
                    nc.tensor.transpose(tp[:, 0:128], kq_sb[:, 0:128], identR[:])
                    nc.tensor.transpose(tp[0:64, 128:256], kq_sb[:, 128:192],
                                        identR[:])
                    nc.tensor.transpose(tp[:, 256:384], kq_sb[:, 192:320],
                                        identR[:])
                    nc.tensor.transpose(tp[0:64, 384:512], kq_sb[:, 320:384],
                                        identR[:])
                    ts_ = slice(i * 128, (i + 1) * 128)
                    nc.scalar.copy(kT01[:, ts_], tp[:, 0:128])
                    nc.vector.tensor_copy(kT2[:, ts_], tp[0:64, 128:256])
                    nc.scalar.copy(qT01[:, ts_], tp[:, 256:384])
                    nc.vector.tensor_copy(qT2[:, ts_], tp[0:64, 384:512])

              # projection weights load during stage 2
              nc.sync.dma_start(wp_sb[:].rearrange("p (n m) -> p n m", n=NCC),
                                wp.rearrange("(n p) m -> p n m", p=128).bitcast(F32R))
              nc.sync.dma_start(bp_sb[:], bp.bitcast(F32R))

              with tc.tile_pool(name="s23ps", bufs=2, space="PSUM") as s2ps:
                  s3ps = s2ps
                  for b, (bs, bw) in enumerate(SPLITS):
                      for h in range(HG):
                          kT = (kT01[0:64], kT01[64:128], kT2[0:64])[h]
                          qT = (qT01[0:64], qT01[64:128], qT2[0:64])[h]
                          oT = (oT01h[b][0:64], oT01h[b][64:128], oT2h[b][0:64])[h]
                          nblk = (bs + bw) // 128
                          o_ps = s2ps.tile([65, TQ], F32, tag="o", bufs=2)

                          # greedy-pack tk chunks (width 512 non-diag, ragged on
                          # the diagonal) into 1024-col score tiles: one exp per
                          # pack, post-exp triangle mask on diagonal chunks
                          chunks, packs, cur, w = [], [], [], 0
                          for t in range(nblk):
                              diag = t * 128 >= bs
                              col0 = t * 128 - bs if diag else 0
                              ncols = bw - col0
                              if w + ncols > 2 * TQ:
                                  packs.append(cur)
                                  cur, w = [], 0
                              cur.append((t, col0, ncols, w, diag))
                              w += ncols
                          if cur:
                              packs.append(cur)
                          for pk in packs:
                              pw = sum(c[2] for c in pk)
                              s_ps = s2ps.tile([128, 2 * TQ], F32, tag="s", bufs=2)
                              wei = s2.tile([128, 2 * TQ], F32R, tag="wei", bufs=4)
                              for t, col0, ncols, off, diag in pk:
                                  nc.tensor.matmul(
                                      s_ps[:, off:off + ncols],
                                      kT[:, t * 128:(t + 1) * 128],
                                      qT[:, bs + col0:bs + bw],
                                      start=True, stop=True)
                              nc.scalar.activation(wei[:, 0:pw], s_ps[:, 0:pw],
                                                   EXP, scale=SCALE)
                              for t, col0, ncols, off, diag in pk:
                                  if diag:
                                      nc.vector.tensor_mul(wei[:, off:off + 128],
                                                           wei[:, off:off + 128],
                                                           triR[:])
                                  va = t * 195 + h * 65
                                  nc.tensor.matmul(
                                      o_ps[:, col0:bw], vaug[:, va:va + 65],
                                      wei[:, off:off + ncols],
                                      start=(t == 0), stop=(t == nblk - 1))
                          recip = s2.tile([1, TQ], F32, tag="recip", bufs=2)
                          nc.vector.reciprocal(recip[:, 0:bw], o_ps[64:65, 0:bw])
                          rb = s2.tile([64, TQ], F32, tag="rb", bufs=2)
                          nc.gpsimd.partition_broadcast(rb[:, 0:bw],
                                                        recip[:, 0:bw])
                          nc.vector.tensor_mul(oT[:], o_ps[0:64, 0:bw],
                                               rb[:, 0:bw])

                  # ---- Stage 3: one AllGather within the batch group,
                  # then the column-sharded projection. A single collective:
                  # on this stack each collective costs ~0.5 ms, so fewer
                  # beats finer overlap.
                  import os as _os
                  _skip_s3 = bool(int(_os.environ.get("BASS_SKIP_S3", "0")))
                  dp = tc.alloc_tile_pool(name="dram", bufs=1, space="DRAM")
                  if _skip_s3:
                      for q, (qbs, qbw) in enumerate(SPLITS):
                          hv = out_t[0:128, qbs // 2:(qbs + qbw) // 2]
                          nc.sync.dma_start(hv.bitcast(F16), oT01h[q][:])
                          lv = out_t[128:CG, qbs // 2:(qbs + qbw) // 2]
                          nc.scalar.dma_start(lv.bitcast(F16), oT2h[q][:])
                  else:
                      ag_in = dp.tile([CG, T], F16)
                      ag_out = dp.tile([G * CG, T], F16)
                      for q, (qbs, qbw) in enumerate(SPLITS):
                          nc.sync.dma_start(
                              ag_in[0:128, qbs:qbs + qbw], oT01h[q][:])
                          nc.scalar.dma_start(
                              ag_in[128:CG, qbs:qbs + qbw], oT2h[q][:])
                      if sim_variant:
                          for gg in range(G):
                              nc.sync.dma_start(
                                  ag_out[gg * CG:(gg + 1) * CG, :], ag_in[:])
                      else:
                          nc.gpsimd.collective_compute(
                              "AllGather", mybir.AluOpType.bypass,
                              replica_groups=[[0, 1, 2, 3], [4, 5, 6, 7]],
                              ins=[ag_in[:].opt()], outs=[ag_out[:].opt()])
                      for j in range(T // TQ):
                          a_bf = s3.tile([128, NCC * TQ], F16, tag="abf",
                                         bufs=2)
                          nc.sync.dma_start(
                              a_bf[:].rearrange("p (n m) -> p n m", n=NCC),
                              ag_out[:, j * TQ:(j + 1) * TQ]
                              .rearrange("(n p) m -> p n m", p=128))
                          a_all = s3.tile([128, NCC * TQ], F32R, tag="aall",
                                          bufs=2)
                          nc.scalar.copy(a_all[:], a_bf[:])
                          p01 = s3ps.tile([128, 2 * TQ], F32, tag="p01",
                                          bufs=1)
                          p0 = p01[:, 0:TQ]
                          p1 = p01[0:64, TQ:2 * TQ]
                          for c in range(NCC):
                              a_sb = a_all[:, c * TQ:(c + 1) * TQ]
                              nc.tensor.matmul(
                                  p0, wp_sb[:, c * CG:c * CG + 128],
                                  a_sb, start=(c == 0), stop=False)
                              nc.tensor.matmul(
                                  p1, wp_sb[:, c * CG + 128:(c + 1) * CG],
                                  a_sb, start=(c == 0), stop=False)
                          nc.tensor.matmul(p0, bp_sb[:, 0:128], ones_sb[:],
                                           start=False, stop=True)
                          nc.tensor.matmul(p1, bp_sb[:, 128:CG], ones_sb[:],
                                           start=False, stop=True)
                          o_all0 = s3.tile([128, TQ], F32, tag="oall0",
                                           bufs=2)
                          o_all1 = s3.tile([64, TQ], F32, tag="oall1",
                                           bufs=2)
                          nc.scalar.copy(o_all0[:], p0)
                          nc.scalar.copy(o_all1[:], p1)
                          qs = slice(j * TQ, (j + 1) * TQ)
                          nc.sync.dma_start(out_t[0:128, qs], o_all0[:])
                          nc.scalar.dma_start(out_t[128:CG, qs], o_all1[:])
def _build(sim_variant=False, reps=1):
    nc = bacc.Bacc("TRN2", target_bir_lowering=False, debug=False,
                   num_devices=1 if sim_variant else N_CORES,
                   enable_asserts=False)
    x = nc.dram_tensor("x", [T, C], F32, kind="ExternalInput").ap()
    wkv = nc.dram_tensor("wkv", [C, 2 * CG], F32, kind="ExternalInput").ap()
    wp = nc.dram_tensor("wp", [C, CG], F32, kind="ExternalInput").ap()
    bp = nc.dram_tensor("bp", [1, CG], F32, kind="ExternalInput").ap()
    cos3 = nc.dram_tensor("cos3", [128, NT * 192], F32, kind="ExternalInput").ap()
    sin3 = nc.dram_tensor("sin3", [128, NT * 192], F32, kind="ExternalInput").ap()
    out_t = nc.dram_tensor("out_t", [CG, T], F32, kind="ExternalOutput").ap()
    with tile.TileContext(nc) as tc:
        _body(nc, tc, x, wkv, wp, bp, cos3, sin3, out_t, sim_variant, reps)
    nc.compile()
    return nc


_NC = None


def _get_nc():
    global _NC
    if _NC is None:
        _NC = _build()
    return _NC


_EXEC = None


def _get_exec():
    global _EXEC
    if _EXEC is None:
        _EXEC = _make_exec(_get_nc())
    return _EXEC


def _make_exec(nc):
    """Reusable jitted SPMD executable (mirrors bass2jax.run_bass_via_pjrt's
    multi-core path)."""
    import jax
    from jax.experimental.shard_map import shard_map
    from jax.sharding import Mesh, PartitionSpec
    from concourse import bass2jax, mybir as _mybir

    bass2jax.install_neuronx_cc_hook()
    in_names, out_names, out_avals, zero_outs = [], [], [], []
    assert nc.dbg_addr is None
    pname = nc.partition_id_tensor.name if nc.partition_id_tensor else None
    for alloc in nc.m.functions[0].allocations:
        if not isinstance(alloc, _mybir.MemoryLocationSet):
            continue
        name = alloc.memorylocations[0].name
        if alloc.kind == "ExternalInput":
            if name != pname:
                in_names.append(name)
        elif alloc.kind == "ExternalOutput":
            out_names.append(name)
            shape = tuple(alloc.tensor_shape)
            dtype = _mybir.dt.np(alloc.dtype)
            out_avals.append(jax.core.ShapedArray(shape, dtype))
            zero_outs.append(np.zeros(shape, dtype))
    n_params = len(in_names)
    all_names = in_names + out_names
    if pname is not None:
        all_names = all_names + [pname]

    def _fn(*args):
        operands = list(args)
        if pname is not None:
            operands.append(bass2jax.partition_id_tensor())
        outs = bass2jax._bass_exec_p.bind(
            *operands,
            out_avals=tuple(out_avals),
            in_names=tuple(all_names),
            out_names=tuple(out_names),
            lowering_input_output_aliases=(),
            sim_require_finite=True,
            sim_require_nnan=True,
            nc=nc,
        )
        return tuple(outs)

    devices = jax.devices()[:N_CORES]
    mesh = Mesh(np.asarray(devices), ("core",))
    nin = n_params + len(out_names)
    donate = tuple(range(n_params, n_params + len(out_names)))
    sharded = jax.jit(
        shard_map(_fn, mesh=mesh,
                  in_specs=(PartitionSpec("core"),) * nin,
                  out_specs=(PartitionSpec("core"),) * len(out_names),
                  check_rep=False),
        donate_argnums=donate, keep_unused=True)

    def _zero_cat():
        return [np.zeros((N_CORES * z.shape[0], *z.shape[1:]), z.dtype)
                for z in zero_outs]

    return (sharded, in_names, out_names, out_avals, _zero_cat)


def _run_cached(in_maps):
    sharded, in_names, out_names, out_avals, zero_cat = _get_exec()
    concat_in = [np.concatenate([np.asarray(in_maps[c][n])
                                 for c in range(N_CORES)], axis=0)
                 for n in in_names]
    out_arrs = sharded(*concat_in, *zero_cat())
    return [
        {name: np.asarray(out_arrs[i]).reshape(N_CORES, *out_avals[i].shape)[c]
         for i, name in enumerate(out_names)}
        for c in range(N_CORES)
    ]


def _prep_rope(r):
    # [T, 32] -> [128, NT*192]: chunk i cols [i*192 + h*64 + 2j, +2j+1] both
    # hold r[i*128+p, j] (duplicated across the channel pair, per head)
    rr = r.reshape(NT, 128, 32).transpose(1, 0, 2)           # [128, NT, 32]
    rr = np.repeat(rr, 2, axis=2)                            # [128, NT, 64]
    rr = np.broadcast_to(rr[:, :, None, :], (128, NT, HG, 64))
    return np.ascontiguousarray(rr.reshape(128, NT * 192), dtype=np.float32)


def _shard_inputs(x, rope_cos, rope_sin, W_att, W_proj, b_proj):
    x = np.ascontiguousarray(np.asarray(x, np.float32))
    W_att = np.asarray(W_att, np.float32)
    W_proj = np.asarray(W_proj, np.float32)
    b_proj = np.asarray(b_proj, np.float32)
    cos3 = _prep_rope(np.asarray(rope_cos, np.float32))
    sin3 = _prep_rope(np.asarray(rope_sin, np.float32))
    in_maps = []
    for r in range(N_CORES):
        b, g = divmod(r, G)
        c0, c1 = g * CG, (g + 1) * CG
        wkv = np.ascontiguousarray(
            np.concatenate([W_att[:, c0:c1],
                            W_att[:, 2 * C + c0:2 * C + c1]], axis=1))
        in_maps.append({
            "x": x[b],
            "wkv": wkv,
            "wp": np.ascontiguousarray(W_proj[:, c0:c1]),
            "bp": np.ascontiguousarray(b_proj[c0:c1][None, :]),
            "cos3": cos3,
            "sin3": sin3,
        })
    return in_maps


def kernel(x, rope_cos, rope_sin, W_att, W_proj, b_proj, _run_kwargs=None):
    nc = _get_nc()
    in_maps = _shard_inputs(x, rope_cos, rope_sin, W_att, W_proj, b_proj)
    global _FIRST_CALL_DONE, _last_in_maps
    _last_in_maps = in_maps
    if not _FIRST_CALL_DONE:
        res = run_bass_kernel_spmd(nc, in_maps, core_ids=list(range(N_CORES)),
                                   **(_run_kwargs or {}))
        results = res.results
        kernel.last_results = res
        _FIRST_CALL_DONE = True
    else:
        results = _run_cached(in_maps)
    out = np.empty((B, T, C), np.float32)
    for r in range(N_CORES):
        b, g = divmod(r, G)
        out[b, :, g * CG:(g + 1) * CG] = results[r]["out_t"].T
    return out


_FIRST_CALL_DONE = False



# revision 9
# speedup vs baseline: 1.2790x; 1.2790x over previous
"""Trainium2 Bass kernel for DecoderMultiHeadAttention (B=2, T=2048, C=768, H=12).

Sharding: 8 cores = 2 batches x 4 head-groups (3 heads each).
Per core: K,V projections for its head group from a host-pre-transposed
x^T (bf16), RoPE in deinterleaved pair layout (host permutes W_att
columns per head to [evens|odds]; W_proj rows permuted to match), causal
flash-style attention with transposed score layout in bf16, AllGather of
f16 attention outputs within each batch group of 4 cores, then a
column-sharded output projection reading the gathered f16 directly.

Note: the reference uses q = rope(v) (faithful source bug), so the
q-chunk of W_att (columns C..2C) is never used and is not computed.
"""

import sys

_REPO = "/opt/trn_rl_repo"
if _REPO not in sys.path:
    sys.path.insert(0, _REPO)

import numpy as np

import concourse.bass as bass
import concourse.mybir as mybir
import concourse.tile as tile
from concourse import bacc
from concourse.bass_utils import run_bass_kernel_spmd
from concourse.masks import make_identity

B, T, C, H = 2, 2048, 768, 12
D = C // H            # 64
N_CORES = 8
G = 4                 # head groups
HG = H // G           # 3 heads per group
CG = HG * D           # 192 output columns per group
NT = T // 128         # 16 t-chunks
NCC = C // 128        # 6 c-chunks
TQ = 512              # q block width
F32 = mybir.dt.float32
F32R = mybir.dt.float32r
BF16 = mybir.dt.bfloat16
F16 = mybir.dt.float16
EXP = mybir.ActivationFunctionType.Exp
SCALE = float(D) ** -0.5


def _body(nc, tc, xt, wkv, wp, bp, cos3, sin3, out_t, sim_variant=False, reps=1):
    with tc.tile_pool(name="const", bufs=1) as cp:
        ident = cp.tile([128, 128], F32)
        make_identity(nc, ident[:])
        identB = cp.tile([128, 128], BF16)
        nc.gpsimd.tensor_copy(identB[:], ident[:])
        # tri[p, f] = 1.0 if f >= p else 0.0  (keep tq >= tk in diagonal blocks)
        tri_f = cp.tile([128, 128], F32)
        nc.gpsimd.memset(tri_f[:], 1.0)
        nc.gpsimd.affine_select(
            out=tri_f[:], in_=tri_f[:], compare_op=mybir.AluOpType.is_ge,
            fill=0.0, base=0, pattern=[[1, 128]], channel_multiplier=-1)
        tri = cp.tile([128, 128], BF16)
        nc.gpsimd.tensor_copy(tri[:], tri_f[:])

        # x^T staged whole in SBUF (bf16, 24KB/partition), loaded in quarters;
        # quarter 0 issued first so the first KV matmul starts ASAP.
        xT_all = cp.tile([128, NCC * T], BF16)
        nc.sync.dma_start(
            xT_all[:].rearrange("p (n m) -> p n m", n=NCC)[:, :, 0:TQ],
            xt[:, 0:TQ].rearrange("(n p) m -> p n m", p=128))
        # weights (host layouts: wkv [128, NCC*2CG] bf16, wp [128, NCC*CG] f16)
        wkv_sb = cp.tile([128, NCC * 2 * CG], BF16)
        nc.scalar.dma_start(wkv_sb[:], wkv)
        for q in range(1, 4):
            nc.sync.dma_start(
                xT_all[:].rearrange("p (n m) -> p n m", n=NCC)
                [:, :, q * TQ:(q + 1) * TQ],
                xt[:, q * TQ:(q + 1) * TQ].rearrange("(n p) m -> p n m", p=128))
        # rope tables, deinterleaved pair layout: per chunk i, cols
        # [i*96 + h*32 + j] = cos/sin(ang[i*128+p, j])
        cos_sb = cp.tile([128, NT * 96], BF16)
        sin_sb = cp.tile([128, NT * 96], BF16)
        nc.scalar.dma_start(cos_sb[:], cos3)
        nc.scalar.dma_start(sin_sb[:], sin3)
        wp_sb = cp.tile([128, NCC * CG], F16)
        nc.scalar.dma_start(wp_sb[:], wp)
        bp_sb = cp.tile([1, CG], F16)
        nc.scalar.dma_start(bp_sb[:], bp)
        ones_f = cp.tile([1, TQ], F16)
        nc.gpsimd.memset(ones_f[:], 1.0)

        # persistent per-head [D, T] tensors (bf16): heads 0,1 packed
        kT01 = cp.tile([128, T], BF16)
        kT2 = cp.tile([64, T], BF16)
        qT01 = cp.tile([128, T], BF16)
        qT2 = cp.tile([64, T], BF16)
        # V in [T, D] layout with a ones column appended per head
        vaug = cp.tile([128, NT * (HG * 65)], BF16)
        ones48 = cp.tile([128, NT * HG], BF16)
        nc.gpsimd.memset(ones48[:], 1.0)
        nc.gpsimd.tensor_copy(
            vaug[:].rearrange("p (k c) -> p k c", c=65)[:, :, 64], ones48[:])
        # attention output, transposed [CG, T] f16, one tile pair per split
        SPLITS = [(0, 512), (512, 512), (1024, 512),
                  (1536, 256), (1792, 256)]
        oT01h = [cp.tile([128, bw], F16, name=f"oT01h{k}")
                 for k, (bs_, bw) in enumerate(SPLITS)]
        oT2h = [cp.tile([64, bw], F16, name=f"oT2h{k}")
                for k, (bs_, bw) in enumerate(SPLITS)]

        for _rep in range(reps):
            # ---- Stage 1: KV projection + RoPE + per-head transposes ----
            with tc.tile_pool(name="s1", bufs=3) as s1, \
                 tc.tile_pool(name="s2", bufs=2) as s2, \
                 tc.tile_pool(name="s3", bufs=1) as s3:
              with tc.tile_pool(name="s1ps", bufs=2, space="PSUM") as s1ps:
                for i in range(NT):
                    kv_ps = s1ps.tile([128, 2 * CG], F32, tag="kv", bufs=2)
                    for c in range(NCC):
                        nc.tensor.matmul(
                            kv_ps[:],
                            xT_all[:, c * T + i * 128:c * T + (i + 1) * 128],
                            wkv_sb[:, c * 2 * CG:(c + 1) * 2 * CG],
                            start=(c == 0), stop=(c == NCC - 1))
                    # K half staged to SBUF bf16 (Pool); V half into vaug (DVE)
                    ksb = s1.tile([128, CG], BF16, tag="ksb", bufs=2)
                    nc.scalar.copy(ksb[:], kv_ps[:, 0:CG])
                    vdst = vaug[:, i * 195:(i + 1) * 195] \
                        .rearrange("p (h c) -> p h c", h=HG)[:, :, 0:64]
                    nc.vector.tensor_copy(
                        vdst, kv_ps[:, CG:2 * CG]
                        .rearrange("p (h c) -> p h c", h=HG))

                    # RoPE in deinterleaved layout: [xr(32)|xi(32)] per head.
                    # or = xr*c - xi*s ; oi = xr*s + xi*c
                    kq = s1.tile([128, 2 * CG], BF16, tag="kq", bufs=2)
                    cS = cos_sb[:, i * 96:(i + 1) * 96] \
                        .rearrange("p (h d) -> p h d", h=HG)
                    sS = sin_sb[:, i * 96:(i + 1) * 96] \
                        .rearrange("p (h d) -> p h d", h=HG)
                    vsrc = vaug[:, i * 195:(i + 1) * 195] \
                        .rearrange("p (h c) -> p h c", h=HG)
                    for half, src in ((0, ksb[:].rearrange(
                            "p (h q d) -> p h q d", h=HG, q=2)),
                            (1, vsrc[:, :, 0:64].rearrange(
                                "p h (q d) -> p h q d", q=2))):
                        xr = src[:, :, 0, :]
                        xi = src[:, :, 1, :]
                        dst = kq[:, half * CG:(half + 1) * CG] \
                            .rearrange("p (h q d) -> p h q d", h=HG, q=2)
                        m1 = s1.tile([128, 96], BF16, tag=f"m1{half}", bufs=2)
                        m2 = s1.tile([128, 96], BF16, tag=f"m2{half}", bufs=2)
                        m3 = s1.tile([128, 96], BF16, tag=f"m3{half}", bufs=2)
                        m4 = s1.tile([128, 96], BF16, tag=f"m4{half}", bufs=2)
                        m1v = m1[:].rearrange("p (h d) -> p h d", h=HG)
                        m2v = m2[:].rearrange("p (h d) -> p h d", h=HG)
                        m3v = m3[:].rearrange("p (h d) -> p h d", h=HG)
                        m4v = m4[:].rearrange("p (h d) -> p h d", h=HG)
                        nc.vector.tensor_mul(m1v, xr, cS)
                        nc.gpsimd.tensor_mul(m2v, xi, sS)
                        nc.vector.tensor_sub(dst[:, :, 0, :], m1v, m2v)
                        nc.vector.tensor_mul(m3v, xr, sS)
                        nc.gpsimd.tensor_mul(m4v, xi, cS)
                        nc.vector.tensor_add(dst[:, :, 1, :], m3v, m4v)

                    # transpose roped K and Q into [D, T] per-head layouts
                    tp = s1ps.tile([128, 512], BF16, tag="tp", bufs=2)
                    nc.tensor.transpose(tp[:, 0:128], kq[:, 0:128], identB[:])
                    nc.tensor.transpose(tp[0:64, 128:256], kq[:, 128:192],
                                        identB[:])
                    nc.tensor.transpose(tp[:, 256:384], kq[:, 192:320],
                                        identB[:])
                    nc.tensor.transpose(tp[0:64, 384:512], kq[:, 320:384],
                                        identB[:])
                    ts_ = slice(i * 128, (i + 1) * 128)
                    nc.scalar.copy(kT01[:, ts_], tp[:, 0:128])
                    nc.scalar.copy(kT2[:, ts_], tp[0:64, 128:256])
                    nc.scalar.copy(qT01[:, ts_], tp[:, 256:384])
                    nc.scalar.copy(qT2[:, ts_], tp[0:64, 384:512])

              with tc.tile_pool(name="s23ps", bufs=2, space="PSUM") as s2ps:
                  s3ps = s2ps
                  dp = tc.alloc_tile_pool(name="dram", bufs=1, space="DRAM")
                  # staged stage 3: per-part AllGather fires as soon as its
                  # splits' oT are done; its projection is emitted one
                  # attention-split later so PE never stalls on the collective
                  PARTS = [((0, 1), (0, 1024)), ((2,), (1024, 1536)),
                           ((3, 4), (1536, 2048))]

                  def attn_split(b):
                      bs, bw = SPLITS[b]
                      for h in range(HG):
                          kT = (kT01[0:64], kT01[64:128], kT2[0:64])[h]
                          qT = (qT01[0:64], qT01[64:128], qT2[0:64])[h]
                          oT = (oT01h[b][0:64], oT01h[b][64:128],
                                oT2h[b][0:64])[h]
                          nblk = (bs + bw) // 128
                          # rows 0:64 = AV, row 64 = Z
                          o_ps = s2ps.tile([128, TQ], F32, tag="o", bufs=2)

                          packs, cur, w = [], [], 0
                          for t in range(nblk):
                              diag = t * 128 >= bs
                              col0 = t * 128 - bs if diag else 0
                              ncols = bw - col0
                              if w + ncols > 2 * TQ:
                                  packs.append(cur)
                                  cur, w = [], 0
                              cur.append((t, col0, ncols, w, diag))
                              w += ncols
                          if cur:
                              packs.append(cur)
                          for pk in packs:
                              pw = sum(c[2] for c in pk)
                              s_ps = s2ps.tile([128, 2 * TQ], F32, tag="s",
                                               bufs=2)
                              wei = s2.tile([128, 2 * TQ], BF16, tag="wei",
                                            bufs=4)
                              for t, col0, ncols, off, diag in pk:
                                  nc.tensor.matmul(
                                      s_ps[:, off:off + ncols],
                                      kT[:, t * 128:(t + 1) * 128],
                                      qT[:, bs + col0:bs + bw],
                                      start=True, stop=True)
                              nc.scalar.activation(wei[:, 0:pw], s_ps[:, 0:pw],
                                                   EXP, scale=SCALE)
                              for t, col0, ncols, off, diag in pk:
                                  if diag:
                                      nc.vector.tensor_mul(
                                          wei[:, off:off + 128],
                                          wei[:, off:off + 128], tri[:])
                                  va = t * 195 + h * 65
                                  nc.tensor.matmul(
                                      o_ps[0:65, col0:bw],
                                      vaug[:, va:va + 65],
                                      wei[:, off:off + ncols],
                                      start=(t == 0), stop=(t == nblk - 1))
                          recip = s2.tile([1, TQ], F32, tag="recip", bufs=2)
                          nc.vector.reciprocal(recip[:, 0:bw],
                                               o_ps[64:65, 0:bw])
                          rb = s2.tile([64, TQ], F32, tag="rb", bufs=2)
                          nc.gpsimd.partition_broadcast(rb[:, 0:bw],
                                                        recip[:, 0:bw])
                          nc.vector.tensor_mul(oT[:], o_ps[0:64, 0:bw],
                                               rb[:, 0:bw])

                  ag_outs = {}

                  def coll_part(splits, c0, c1):
                      w = c1 - c0
                      ag_in = dp.tile([CG, w], F16, name=f"agi{c0}")
                      ag_out = dp.tile([G * CG, w], F16, name=f"ago{c0}")
                      ag_outs[c0] = ag_out
                      for q in splits:
                          qbs, qbw = SPLITS[q]
                          nc.sync.dma_start(
                              ag_in[0:128, qbs - c0:qbs - c0 + qbw],
                              oT01h[q][:])
                          nc.sync.dma_start(
                              ag_in[128:CG, qbs - c0:qbs - c0 + qbw],
                              oT2h[q][:])
                      if sim_variant:
                          for gg in range(G):
                              nc.sync.dma_start(
                                  ag_out[gg * CG:(gg + 1) * CG, :], ag_in[:])
                      else:
                          nc.gpsimd.collective_compute(
                              "AllGather", mybir.AluOpType.bypass,
                              replica_groups=[[0, 1, 2, 3], [4, 5, 6, 7]],
                              ins=[ag_in[:].opt()], outs=[ag_out[:].opt()])

                  def proj_part(c0, c1):
                      ag_out = ag_outs[c0]
                      for j in range(c0 // TQ, c1 // TQ):
                          a_bf = s3.tile([128, NCC * TQ], F16, tag="abf",
                                         bufs=2)
                          nc.sync.dma_start(
                              a_bf[:].rearrange("p (n m) -> p n m", n=NCC),
                              ag_out[:, j * TQ - c0:(j + 1) * TQ - c0]
                              .rearrange("(n p) m -> p n m", p=128))
                          p01 = s3ps.tile([128, 2 * TQ], F32, tag="p01",
                                          bufs=1)
                          p0 = p01[:, 0:TQ]
                          p1 = p01[0:64, TQ:2 * TQ]
                          for c in range(NCC):
                              a_sb = a_bf[:, c * TQ:(c + 1) * TQ]
                              nc.tensor.matmul(
                                  p0, wp_sb[:, c * CG:c * CG + 128],
                                  a_sb, start=(c == 0), stop=False)
                              nc.tensor.matmul(
                                  p1, wp_sb[:, c * CG + 128:(c + 1) * CG],
                                  a_sb, start=(c == 0), stop=False)
                          nc.tensor.matmul(p0, bp_sb[:, 0:128], ones_f[:],
                                           start=False, stop=True)
                          nc.tensor.matmul(p1, bp_sb[:, 128:CG], ones_f[:],
                                           start=False, stop=True)
                          o_all0 = s3.tile([128, TQ], F32, tag="oall0",
                                           bufs=2)
                          o_all1 = s3.tile([64, TQ], F32, tag="oall1",
                                           bufs=2)
                          nc.vector.tensor_copy(o_all0[:], p0)
                          nc.scalar.copy(o_all1[:], p1)
                          qs = slice(j * TQ, (j + 1) * TQ)
                          nc.sync.dma_start(out_t[0:128, qs], o_all0[:])
                          nc.sync.dma_start(out_t[128:CG, qs], o_all1[:])

                  attn_split(0)
                  attn_split(1)
                  coll_part(*PARTS[0][0:1], *PARTS[0][1])
                  attn_split(2)
                  proj_part(*PARTS[0][1])
                  coll_part(*PARTS[1][0:1], *PARTS[1][1])
                  attn_split(3)
                  proj_part(*PARTS[1][1])
                  attn_split(4)
                  coll_part(*PARTS[2][0:1], *PARTS[2][1])
                  proj_part(*PARTS[2][1])


def _build(sim_variant=False, reps=1):
    nc = bacc.Bacc("TRN2", target_bir_lowering=False, debug=False,
                   num_devices=1 if sim_variant else N_CORES,
                   enable_asserts=False)
    xt = nc.dram_tensor("xt", [C, T], BF16, kind="ExternalInput").ap()
    wkv = nc.dram_tensor("wkv", [128, NCC * 2 * CG], BF16,
                         kind="ExternalInput").ap()
    wp = nc.dram_tensor("wp", [128, NCC * CG], F16, kind="ExternalInput").ap()
    bp = nc.dram_tensor("bp", [1, CG], F16, kind="ExternalInput").ap()
    cos3 = nc.dram_tensor("cos3", [128, NT * 96], BF16,
                          kind="ExternalInput").ap()
    sin3 = nc.dram_tensor("sin3", [128, NT * 96], BF16,
                          kind="ExternalInput").ap()
    out_t = nc.dram_tensor("out_t", [CG, T], F32, kind="ExternalOutput").ap()
    with tile.TileContext(nc) as tc:
        _body(nc, tc, xt, wkv, wp, bp, cos3, sin3, out_t, sim_variant, reps)
    nc.compile()
    return nc


_NC = None


def _get_nc():
    global _NC
    if _NC is None:
        _NC = _build()
    return _NC


_EXEC = None


def _get_exec():
    global _EXEC
    if _EXEC is None:
        _EXEC = _make_exec(_get_nc())
    return _EXEC


def _make_exec(nc):
    """Reusable jitted SPMD executable (mirrors bass2jax.run_bass_via_pjrt's
    multi-core path)."""
    import jax
    from jax.experimental.shard_map import shard_map
    from jax.sharding import Mesh, PartitionSpec
    from concourse import bass2jax, mybir as _mybir

    bass2jax.install_neuronx_cc_hook()
    in_names, out_names, out_avals, zero_outs = [], [], [], []
    assert nc.dbg_addr is None
    pname = nc.partition_id_tensor.name if nc.partition_id_tensor else None
    for alloc in nc.m.functions[0].allocations:
        if not isinstance(alloc, _mybir.MemoryLocationSet):
            continue
        name = alloc.memorylocations[0].name
        if alloc.kind == "ExternalInput":
            if name != pname:
                in_names.append(name)
        elif alloc.kind == "ExternalOutput":
            out_names.append(name)
            shape = tuple(alloc.tensor_shape)
            dtype = _mybir.dt.np(alloc.dtype)
            out_avals.append(jax.core.ShapedArray(shape, dtype))
            zero_outs.append(np.zeros(shape, dtype))
    n_params = len(in_names)
    all_names = in_names + out_names
    if pname is not None:
        all_names = all_names + [pname]

    def _fn(*args):
        operands = list(args)
        if pname is not None:
            operands.append(bass2jax.partition_id_tensor())
        outs = bass2jax._bass_exec_p.bind(
            *operands,
            out_avals=tuple(out_avals),
            in_names=tuple(all_names),
            out_names=tuple(out_names),
            lowering_input_output_aliases=(),
            sim_require_finite=True,
            sim_require_nnan=True,
            nc=nc,
        )
        return tuple(outs)

    devices = jax.devices()[:N_CORES]
    mesh = Mesh(np.asarray(devices), ("core",))
    nin = n_params + len(out_names)
    donate = tuple(range(n_params, n_params + len(out_names)))
    sharded = jax.jit(
        shard_map(_fn, mesh=mesh,
                  in_specs=(PartitionSpec("core"),) * nin,
                  out_specs=(PartitionSpec("core"),) * len(out_names),
                  check_rep=False),
        donate_argnums=donate, keep_unused=True)

    def _zero_cat():
        return [np.zeros((N_CORES * z.shape[0], *z.shape[1:]), z.dtype)
                for z in zero_outs]

    return (sharded, in_names, out_names, out_avals, _zero_cat)


def _run_cached(in_maps):
    sharded, in_names, out_names, out_avals, zero_cat = _get_exec()
    concat_in = [np.concatenate([np.asarray(in_maps[c][n])
                                 for c in range(N_CORES)], axis=0)
                 for n in in_names]
    out_arrs = sharded(*concat_in, *zero_cat())
    return [
        {name: np.asarray(out_arrs[i]).reshape(N_CORES, *out_avals[i].shape)[c]
         for i, name in enumerate(out_names)}
        for c in range(N_CORES)
    ]


_PERM = np.concatenate([np.arange(0, 64, 2), np.arange(1, 64, 2)])


def _prep_rope(r, np_bf16):
    # [T, 32] -> [128, NT*96]: chunk i, cols i*96 + h*32 + j = r[i*128+p, j]
    rr = r.reshape(NT, 128, 32).transpose(1, 0, 2)           # [128, NT, 32]
    rr = np.broadcast_to(rr[:, :, None, :], (128, NT, HG, 32))
    return np.ascontiguousarray(rr.reshape(128, NT * 96)).astype(np_bf16)


def _cm(w, cols):
    # [768, cols] -> c-chunk-major [128, NCC*cols]
    return np.ascontiguousarray(
        w.reshape(NCC, 128, -1).transpose(1, 0, 2).reshape(128, -1))


def _shard_inputs(x, rope_cos, rope_sin, W_att, W_proj, b_proj):
    np_bf16 = mybir.dt.np(BF16)
    x = np.asarray(x, np.float32)
    W_att = np.asarray(W_att, np.float32)
    W_proj = np.asarray(W_proj, np.float32)
    b_proj = np.asarray(b_proj, np.float32)
    cos3 = _prep_rope(np.asarray(rope_cos, np.float32), np_bf16)
    sin3 = _prep_rope(np.asarray(rope_sin, np.float32), np_bf16)
    # W_proj rows permuted to match deinterleaved V columns (all heads)
    rowp = np.concatenate([h * 64 + _PERM for h in range(H)])
    wp_perm = W_proj[rowp]
    xts = [np.ascontiguousarray(x[b].T).astype(np_bf16) for b in range(B)]
    in_maps = []
    for r in range(N_CORES):
        b, g = divmod(r, G)
        heads = [3 * g + h for h in range(HG)]
        kcols = np.concatenate([h * 64 + _PERM for h in heads])
        wkv = np.concatenate(
            [W_att[:, kcols], W_att[:, 2 * C + kcols]], axis=1)  # [768, 384]
        in_maps.append({
            "xt": xts[b],
            "wkv": _cm(wkv, 2 * CG).astype(np_bf16),
            "wp": _cm(wp_perm[:, g * CG:(g + 1) * CG], CG).astype(np.float16),
            "bp": np.ascontiguousarray(
                b_proj[g * CG:(g + 1) * CG][None, :]).astype(np.float16),
            "cos3": cos3,
            "sin3": sin3,
        })
    return in_maps


def kernel(x, rope_cos, rope_sin, W_att, W_proj, b_proj, _run_kwargs=None):
    nc = _get_nc()
    in_maps = _shard_inputs(x, rope_cos, rope_sin, W_att, W_proj, b_proj)
    global _FIRST_CALL_DONE, _last_in_maps
    _last_in_maps = in_maps
    if not _FIRST_CALL_DONE:
        res = run_bass_kernel_spmd(nc, in_maps, core_ids=list(range(N_CORES)),
                                   **(_run_kwargs or {}))
        results = res.results
        kernel.last_results = res
        _FIRST_CALL_DONE = True
    else:
        results = _run_cached(in_maps)
    out = np.empty((B, T, C), np.float32)
    for r in range(N_CORES):
        b, g = divmod(r, G)
        out[b, :, g * CG:(g + 1) * CG] = results[r]["out_t"].T
    return out


_FIRST_CALL_DONE = False


# revision 10
# speedup vs baseline: 1.3245x; 1.0356x over previous
"""Trainium2 Bass kernel for DecoderMultiHeadAttention (B=2, T=2048, C=768, H=12).

Sharding: 8 cores = 2 batches x 4 head-groups (3 heads each).
Per core: K,V projections for its head group from a host-pre-transposed
x^T (bf16), RoPE in deinterleaved pair layout (host permutes W_att
columns per head to [evens|odds]; W_proj rows permuted to match), causal
flash-style attention with transposed score layout in bf16, AllGather of
f16 attention outputs within each batch group of 4 cores, then a
column-sharded output projection reading the gathered f16 directly.

Note: the reference uses q = rope(v) (faithful source bug), so the
q-chunk of W_att (columns C..2C) is never used and is not computed.
"""

import sys

_REPO = "/opt/trn_rl_repo"
if _REPO not in sys.path:
    sys.path.insert(0, _REPO)

import numpy as np

import concourse.bass as bass
import concourse.mybir as mybir
import concourse.tile as tile
from concourse import bacc
from concourse.bass_utils import run_bass_kernel_spmd
from concourse.masks import make_identity

B, T, C, H = 2, 2048, 768, 12
D = C // H            # 64
N_CORES = 8
G = 4                 # head groups
HG = H // G           # 3 heads per group
CG = HG * D           # 192 output columns per group
NT = T // 128         # 16 t-chunks
NCC = C // 128        # 6 c-chunks
TQ = 512              # q block width
F32 = mybir.dt.float32
F32R = mybir.dt.float32r
BF16 = mybir.dt.bfloat16
F16 = mybir.dt.float16
EXP = mybir.ActivationFunctionType.Exp
SCALE = float(D) ** -0.5


def _body(nc, tc, xt, wkv, wp, bp, cos3, sin3, out_t, sim_variant=False, reps=1):
    with tc.tile_pool(name="const", bufs=1) as cp:
        ident = cp.tile([128, 128], F32)
        make_identity(nc, ident[:])
        identB = cp.tile([128, 128], BF16)
        nc.gpsimd.tensor_copy(identB[:], ident[:])
        # tri[p, f] = 1.0 if f >= p else 0.0  (keep tq >= tk in diagonal blocks)
        tri_f = cp.tile([128, 128], F32)
        nc.gpsimd.memset(tri_f[:], 1.0)
        nc.gpsimd.affine_select(
            out=tri_f[:], in_=tri_f[:], compare_op=mybir.AluOpType.is_ge,
            fill=0.0, base=0, pattern=[[1, 128]], channel_multiplier=-1)
        tri = cp.tile([128, 128], BF16)
        nc.gpsimd.tensor_copy(tri[:], tri_f[:])

        # x^T staged whole in SBUF (bf16, 24KB/partition), loaded in quarters;
        # quarter 0 issued first so the first KV matmul starts ASAP.
        xT_all = cp.tile([128, NCC * T], BF16)
        nc.sync.dma_start(
            xT_all[:].rearrange("p (n m) -> p n m", n=NCC)[:, :, 0:TQ],
            xt[:, 0:TQ].rearrange("(n p) m -> p n m", p=128))
        # weights (host layouts: wkv [128, NCC*2CG] bf16, wp [128, NCC*CG] f16)
        wkv_sb = cp.tile([128, NCC * 2 * CG], BF16)
        nc.scalar.dma_start(wkv_sb[:], wkv)
        for q in range(1, 4):
            nc.sync.dma_start(
                xT_all[:].rearrange("p (n m) -> p n m", n=NCC)
                [:, :, q * TQ:(q + 1) * TQ],
                xt[:, q * TQ:(q + 1) * TQ].rearrange("(n p) m -> p n m", p=128))
        # rope tables, deinterleaved pair layout: per chunk i, cols
        # [i*96 + h*32 + j] = cos/sin(ang[i*128+p, j])
        cos_sb = cp.tile([128, NT * 96], BF16)
        sin_sb = cp.tile([128, NT * 96], BF16)
        nc.scalar.dma_start(cos_sb[:], cos3)
        nc.scalar.dma_start(sin_sb[:], sin3)
        wp_sb = cp.tile([128, NCC * CG], F16)
        nc.scalar.dma_start(wp_sb[:], wp)
        bp_sb = cp.tile([1, CG], F16)
        nc.scalar.dma_start(bp_sb[:], bp)
        ones_f = cp.tile([1, TQ], F16)
        nc.gpsimd.memset(ones_f[:], 1.0)

        # persistent per-head [D, T] tensors (bf16): heads 0,1 packed
        kT01 = cp.tile([128, T], BF16)
        kT2 = cp.tile([64, T], BF16)
        qT01 = cp.tile([128, T], BF16)
        qT2 = cp.tile([64, T], BF16)
        # V in [T, D] layout with a ones column appended per head
        vaug = cp.tile([128, NT * (HG * 65)], BF16)
        ones48 = cp.tile([128, NT * HG], BF16)
        nc.gpsimd.memset(ones48[:], 1.0)
        nc.gpsimd.tensor_copy(
            vaug[:].rearrange("p (k c) -> p k c", c=65)[:, :, 64], ones48[:])
        # attention output, transposed [CG, T] f16, one tile pair per split
        SPLITS = [(0, 512), (512, 512), (1024, 512),
                  (1536, 256), (1792, 256)]
        oT01h = [cp.tile([128, bw], F16, name=f"oT01h{k}")
                 for k, (bs_, bw) in enumerate(SPLITS)]
        oT2h = [cp.tile([64, bw], F16, name=f"oT2h{k}")
                for k, (bs_, bw) in enumerate(SPLITS)]

        for _rep in range(reps):
            # ---- Stage 1: KV projection + RoPE + per-head transposes ----
            with tc.tile_pool(name="s1", bufs=3) as s1, \
                 tc.tile_pool(name="s2", bufs=2) as s2, \
                 tc.tile_pool(name="s3", bufs=1) as s3:
              with tc.tile_pool(name="s1ps", bufs=2, space="PSUM") as s1ps:
                for i in range(NT):
                    kv_ps = s1ps.tile([128, 2 * CG], F32, tag="kv", bufs=2)
                    for c in range(NCC):
                        nc.tensor.matmul(
                            kv_ps[:],
                            xT_all[:, c * T + i * 128:c * T + (i + 1) * 128],
                            wkv_sb[:, c * 2 * CG:(c + 1) * 2 * CG],
                            start=(c == 0), stop=(c == NCC - 1))
                    # K half staged to SBUF bf16 (Pool); V half into vaug (DVE)
                    ksb = s1.tile([128, CG], BF16, tag="ksb", bufs=2)
                    nc.scalar.copy(ksb[:], kv_ps[:, 0:CG])
                    vdst = vaug[:, i * 195:(i + 1) * 195] \
                        .rearrange("p (h c) -> p h c", h=HG)[:, :, 0:64]
                    nc.vector.tensor_copy(
                        vdst, kv_ps[:, CG:2 * CG]
                        .rearrange("p (h c) -> p h c", h=HG))

                    # RoPE in deinterleaved layout: [xr(32)|xi(32)] per head.
                    # or = xr*c - xi*s ; oi = xr*s + xi*c
                    kq = s1.tile([128, 2 * CG], BF16, tag="kq", bufs=2)
                    cS = cos_sb[:, i * 96:(i + 1) * 96] \
                        .rearrange("p (h d) -> p h d", h=HG)
                    sS = sin_sb[:, i * 96:(i + 1) * 96] \
                        .rearrange("p (h d) -> p h d", h=HG)
                    vsrc = vaug[:, i * 195:(i + 1) * 195] \
                        .rearrange("p (h c) -> p h c", h=HG)
                    for half, src in ((0, ksb[:].rearrange(
                            "p (h q d) -> p h q d", h=HG, q=2)),
                            (1, vsrc[:, :, 0:64].rearrange(
                                "p h (q d) -> p h q d", q=2))):
                        xr = src[:, :, 0, :]
                        xi = src[:, :, 1, :]
                        dst = kq[:, half * CG:(half + 1) * CG] \
                            .rearrange("p (h q d) -> p h q d", h=HG, q=2)
                        m1 = s1.tile([128, 96], BF16, tag=f"m1{half}", bufs=2)
                        m2 = s1.tile([128, 96], BF16, tag=f"m2{half}", bufs=2)
                        m3 = s1.tile([128, 96], BF16, tag=f"m3{half}", bufs=2)
                        m4 = s1.tile([128, 96], BF16, tag=f"m4{half}", bufs=2)
                        m1v = m1[:].rearrange("p (h d) -> p h d", h=HG)
                        m2v = m2[:].rearrange("p (h d) -> p h d", h=HG)
                        m3v = m3[:].rearrange("p (h d) -> p h d", h=HG)
                        m4v = m4[:].rearrange("p (h d) -> p h d", h=HG)
                        nc.vector.tensor_mul(m1v, xr, cS)
                        nc.gpsimd.tensor_mul(m2v, xi, sS)
                        nc.vector.tensor_sub(dst[:, :, 0, :], m1v, m2v)
                        nc.vector.tensor_mul(m3v, xr, sS)
                        nc.gpsimd.tensor_mul(m4v, xi, cS)
                        nc.vector.tensor_add(dst[:, :, 1, :], m3v, m4v)

                    # transpose roped K and Q into [D, T] per-head layouts
                    tp = s1ps.tile([128, 512], BF16, tag="tp", bufs=2)
                    nc.tensor.transpose(tp[:, 0:128], kq[:, 0:128], identB[:])
                    nc.tensor.transpose(tp[0:64, 128:256], kq[:, 128:192],
                                        identB[:])
                    nc.tensor.transpose(tp[:, 256:384], kq[:, 192:320],
                                        identB[:])
                    nc.tensor.transpose(tp[0:64, 384:512], kq[:, 320:384],
                                        identB[:])
                    ts_ = slice(i * 128, (i + 1) * 128)
                    nc.scalar.copy(kT01[:, ts_], tp[:, 0:128])
                    nc.scalar.copy(kT2[:, ts_], tp[0:64, 128:256])
                    nc.scalar.copy(qT01[:, ts_], tp[:, 256:384])
                    nc.scalar.copy(qT2[:, ts_], tp[0:64, 384:512])

              with tc.tile_pool(name="s23ps", bufs=2, space="PSUM") as s2ps:
                  s3ps = s2ps
                  dp = tc.alloc_tile_pool(name="dram", bufs=1, space="DRAM")
                  # staged stage 3: per-part AllGather fires as soon as its
                  # splits' oT are done; its projection is emitted one
                  # attention-split later so PE never stalls on the collective
                  PARTS = [((0, 1), (0, 1024)), ((2,), (1024, 1536)),
                           ((3,), (1536, 1792)), ((4,), (1792, 2048))]

                  def attn_split(b):
                      bs, bw = SPLITS[b]
                      for h in range(HG):
                          kT = (kT01[0:64], kT01[64:128], kT2[0:64])[h]
                          qT = (qT01[0:64], qT01[64:128], qT2[0:64])[h]
                          oT = (oT01h[b][0:64], oT01h[b][64:128],
                                oT2h[b][0:64])[h]
                          nblk = (bs + bw) // 128
                          # rows 0:64 = AV, row 64 = Z
                          o_ps = s2ps.tile([128, TQ], F32, tag="o", bufs=2)

                          packs, cur, w = [], [], 0
                          for t in range(nblk):
                              diag = t * 128 >= bs
                              col0 = t * 128 - bs if diag else 0
                              ncols = bw - col0
                              if w + ncols > 2 * TQ:
                                  packs.append(cur)
                                  cur, w = [], 0
                              cur.append((t, col0, ncols, w, diag))
                              w += ncols
                          if cur:
                              packs.append(cur)
                          for pk in packs:
                              pw = sum(c[2] for c in pk)
                              s_ps = s2ps.tile([128, 2 * TQ], F32, tag="s",
                                               bufs=2)
                              wei = s2.tile([128, 2 * TQ], BF16, tag="wei",
                                            bufs=4)
                              for t, col0, ncols, off, diag in pk:
                                  nc.tensor.matmul(
                                      s_ps[:, off:off + ncols],
                                      kT[:, t * 128:(t + 1) * 128],
                                      qT[:, bs + col0:bs + bw],
                                      start=True, stop=True)
                              nc.scalar.activation(wei[:, 0:pw], s_ps[:, 0:pw],
                                                   EXP, scale=SCALE)
                              for t, col0, ncols, off, diag in pk:
                                  if diag:
                                      nc.vector.tensor_mul(
                                          wei[:, off:off + 128],
                                          wei[:, off:off + 128], tri[:])
                                  va = t * 195 + h * 65
                                  nc.tensor.matmul(
                                      o_ps[0:65, col0:bw],
                                      vaug[:, va:va + 65],
                                      wei[:, off:off + ncols],
                                      start=(t == 0), stop=(t == nblk - 1))
                          recip = s2.tile([1, TQ], F32, tag="recip", bufs=2)
                          nc.vector.reciprocal(recip[:, 0:bw],
                                               o_ps[64:65, 0:bw])
                          rb = s2.tile([64, TQ], F32, tag="rb", bufs=2)
                          nc.gpsimd.partition_broadcast(rb[:, 0:bw],
                                                        recip[:, 0:bw])
                          nc.vector.tensor_mul(oT[:], o_ps[0:64, 0:bw],
                                               rb[:, 0:bw])

                  ag_outs = {}

                  def coll_part(splits, c0, c1):
                      w = c1 - c0
                      ag_in = dp.tile([CG, w], F16, name=f"agi{c0}")
                      ag_out = dp.tile([G * CG, w], F16, name=f"ago{c0}")
                      ag_outs[c0] = ag_out
                      for q in splits:
                          qbs, qbw = SPLITS[q]
                          nc.sync.dma_start(
                              ag_in[0:128, qbs - c0:qbs - c0 + qbw],
                              oT01h[q][:])
                          nc.sync.dma_start(
                              ag_in[128:CG, qbs - c0:qbs - c0 + qbw],
                              oT2h[q][:])
                      if sim_variant:
                          for gg in range(G):
                              nc.sync.dma_start(
                                  ag_out[gg * CG:(gg + 1) * CG, :], ag_in[:])
                      else:
                          nc.gpsimd.collective_compute(
                              "AllGather", mybir.AluOpType.bypass,
                              replica_groups=[[0, 1, 2, 3], [4, 5, 6, 7]],
                              ins=[ag_in[:].opt()], outs=[ag_out[:].opt()])

                  def proj_part(c0, c1):
                      ag_out = ag_outs[c0]
                      pos = c0
                      while pos < c1:
                          w = min(TQ, c1 - pos)
                          a_bf = s3.tile([128, NCC * TQ], F16, tag="abf",
                                         bufs=2)
                          nc.sync.dma_start(
                              a_bf[:].rearrange("p (n m) -> p n m", n=NCC)
                              [:, :, 0:w],
                              ag_out[:, pos - c0:pos - c0 + w]
                              .rearrange("(n p) m -> p n m", p=128))
                          p01 = s3ps.tile([128, 2 * TQ], F32, tag="p01",
                                          bufs=1)
                          p0 = p01[:, 0:w]
                          p1 = p01[0:64, TQ:TQ + w]
                          for c in range(NCC):
                              a_sb = a_bf[:, c * TQ:c * TQ + w]
                              nc.tensor.matmul(
                                  p0, wp_sb[:, c * CG:c * CG + 128],
                                  a_sb, start=(c == 0), stop=False)
                              nc.tensor.matmul(
                                  p1, wp_sb[:, c * CG + 128:(c + 1) * CG],
                                  a_sb, start=(c == 0), stop=False)
                          nc.tensor.matmul(p0, bp_sb[:, 0:128],
                                           ones_f[:, 0:w],
                                           start=False, stop=True)
                          nc.tensor.matmul(p1, bp_sb[:, 128:CG],
                                           ones_f[:, 0:w],
                                           start=False, stop=True)
                          o_all0 = s3.tile([128, TQ], F32, tag="oall0",
                                           bufs=2)
                          o_all1 = s3.tile([64, TQ], F32, tag="oall1",
                                           bufs=2)
                          nc.vector.tensor_copy(o_all0[:, 0:w], p0)
                          nc.scalar.copy(o_all1[:, 0:w], p1)
                          qs = slice(pos, pos + w)
                          nc.sync.dma_start(out_t[0:128, qs],
                                            o_all0[:, 0:w])
                          nc.sync.dma_start(out_t[128:CG, qs],
                                            o_all1[:, 0:w])
                          pos += w

                  attn_split(0)
                  attn_split(1)
                  coll_part(*PARTS[0][0:1], *PARTS[0][1])
                  attn_split(2)
                  proj_part(*PARTS[0][1])
                  coll_part(*PARTS[1][0:1], *PARTS[1][1])
                  attn_split(3)
                  proj_part(*PARTS[1][1])
                  coll_part(*PARTS[2][0:1], *PARTS[2][1])
                  attn_split(4)
                  proj_part(*PARTS[2][1])
                  coll_part(*PARTS[3][0:1], *PARTS[3][1])
                  proj_part(*PARTS[3][1])


def _build(sim_variant=False, reps=1):
    nc = bacc.Bacc("TRN2", target_bir_lowering=False, debug=False,
                   num_devices=1 if sim_variant else N_CORES,
                   enable_asserts=False)
    xt = nc.dram_tensor("xt", [C, T], BF16, kind="ExternalInput").ap()
    wkv = nc.dram_tensor("wkv", [128, NCC * 2 * CG], BF16,
                         kind="ExternalInput").ap()
    wp = nc.dram_tensor("wp", [128, NCC * CG], F16, kind="ExternalInput").ap()
    bp = nc.dram_tensor("bp", [1, CG], F16, kind="ExternalInput").ap()
    cos3 = nc.dram_tensor("cos3", [128, NT * 96], BF16,
                          kind="ExternalInput").ap()
    sin3 = nc.dram_tensor("sin3", [128, NT * 96], BF16,
                          kind="ExternalInput").ap()
    out_t = nc.dram_tensor("out_t", [CG, T], F32, kind="ExternalOutput").ap()
    with tile.TileContext(nc) as tc:
        _body(nc, tc, xt, wkv, wp, bp, cos3, sin3, out_t, sim_variant, reps)
    nc.compile()
    return nc


_NC = None


def _get_nc():
    global _NC
    if _NC is None:
        _NC = _build()
    return _NC


_EXEC = None


def _get_exec():
    global _EXEC
    if _EXEC is None:
        _EXEC = _make_exec(_get_nc())
    return _EXEC


def _make_exec(nc):
    """Reusable jitted SPMD executable (mirrors bass2jax.run_bass_via_pjrt's
    multi-core path)."""
    import jax
    from jax.experimental.shard_map import shard_map
    from jax.sharding import Mesh, PartitionSpec
    from concourse import bass2jax, mybir as _mybir

    bass2jax.install_neuronx_cc_hook()
    in_names, out_names, out_avals, zero_outs = [], [], [], []
    assert nc.dbg_addr is None
    pname = nc.partition_id_tensor.name if nc.partition_id_tensor else None
    for alloc in nc.m.functions[0].allocations:
        if not isinstance(alloc, _mybir.MemoryLocationSet):
            continue
        name = alloc.memorylocations[0].name
        if alloc.kind == "ExternalInput":
            if name != pname:
                in_names.append(name)
        elif alloc.kind == "ExternalOutput":
            out_names.append(name)
            shape = tuple(alloc.tensor_shape)
            dtype = _mybir.dt.np(alloc.dtype)
            out_avals.append(jax.core.ShapedArray(shape, dtype))
            zero_outs.append(np.zeros(shape, dtype))
    n_params = len(in_names)
    all_names = in_names + out_names
    if pname is not None:
        all_names = all_names + [pname]

    def _fn(*args):
        operands = list(args)
        if pname is not None:
            operands.append(bass2jax.partition_id_tensor())
        outs = bass2jax._bass_exec_p.bind(
            *operands,
            out_avals=tuple(out_avals),
            in_names=tuple(all_names),
            out_names=tuple(out_names),
            lowering_input_output_aliases=(),
            sim_require_finite=True,
            sim_require_nnan=True,
            nc=nc,
        )
        return tuple(outs)

    devices = jax.devices()[:N_CORES]
    mesh = Mesh(np.asarray(devices), ("core",))
    nin = n_params + len(out_names)
    donate = tuple(range(n_params, n_params + len(out_names)))
    sharded = jax.jit(
        shard_map(_fn, mesh=mesh,
                  in_specs=(PartitionSpec("core"),) * nin,
                  out_specs=(PartitionSpec("core"),) * len(out_names),
                  check_rep=False),
        donate_argnums=donate, keep_unused=True)

    def _zero_cat():
        return [np.zeros((N_CORES * z.shape[0], *z.shape[1:]), z.dtype)
                for z in zero_outs]

    return (sharded, in_names, out_names, out_avals, _zero_cat)


def _run_cached(in_maps):
    sharded, in_names, out_names, out_avals, zero_cat = _get_exec()
    concat_in = [np.concatenate([np.asarray(in_maps[c][n])
                                 for c in range(N_CORES)], axis=0)
                 for n in in_names]
    out_arrs = sharded(*concat_in, *zero_cat())
    return [
        {name: np.asarray(out_arrs[i]).reshape(N_CORES, *out_avals[i].shape)[c]
         for i, name in enumerate(out_names)}
        for c in range(N_CORES)
    ]


_PERM = np.concatenate([np.arange(0, 64, 2), np.arange(1, 64, 2)])


def _prep_rope(r, np_bf16):
    # [T, 32] -> [128, NT*96]: chunk i, cols i*96 + h*32 + j = r[i*128+p, j]
    rr = r.reshape(NT, 128, 32).transpose(1, 0, 2)           # [128, NT, 32]
    rr = np.broadcast_to(rr[:, :, None, :], (128, NT, HG, 32))
    return np.ascontiguousarray(rr.reshape(128, NT * 96)).astype(np_bf16)


def _cm(w, cols):
    # [768, cols] -> c-chunk-major [128, NCC*cols]
    return np.ascontiguousarray(
        w.reshape(NCC, 128, -1).transpose(1, 0, 2).reshape(128, -1))


def _shard_inputs(x, rope_cos, rope_sin, W_att, W_proj, b_proj):
    np_bf16 = mybir.dt.np(BF16)
    x = np.asarray(x, np.float32)
    W_att = np.asarray(W_att, np.float32)
    W_proj = np.asarray(W_proj, np.float32)
    b_proj = np.asarray(b_proj, np.float32)
    cos3 = _prep_rope(np.asarray(rope_cos, np.float32), np_bf16)
    sin3 = _prep_rope(np.asarray(rope_sin, np.float32), np_bf16)
    # W_proj rows permuted to match deinterleaved V columns (all heads)
    rowp = np.concatenate([h * 64 + _PERM for h in range(H)])
    wp_perm = W_proj[rowp]
    xts = [np.ascontiguousarray(x[b].T).astype(np_bf16) for b in range(B)]
    in_maps = []
    for r in range(N_CORES):
        b, g = divmod(r, G)
        heads = [3 * g + h for h in range(HG)]
        kcols = np.concatenate([h * 64 + _PERM for h in heads])
        wkv = np.concatenate(
            [W_att[:, kcols], W_att[:, 2 * C + kcols]], axis=1)  # [768, 384]
        in_maps.append({
            "xt": xts[b],
            "wkv": _cm(wkv, 2 * CG).astype(np_bf16),
            "wp": _cm(wp_perm[:, g * CG:(g + 1) * CG], CG).astype(np.float16),
            "bp": np.ascontiguousarray(
                b_proj[g * CG:(g + 1) * CG][None, :]).astype(np.float16),
            "cos3": cos3,
            "sin3": sin3,
        })
    return in_maps


def kernel(x, rope_cos, rope_sin, W_att, W_proj, b_proj, _run_kwargs=None):
    nc = _get_nc()
    in_maps = _shard_inputs(x, rope_cos, rope_sin, W_att, W_proj, b_proj)
    global _FIRST_CALL_DONE, _last_in_maps
    _last_in_maps = in_maps
    if not _FIRST_CALL_DONE:
        res = run_bass_kernel_spmd(nc, in_maps, core_ids=list(range(N_CORES)),
                                   **(_run_kwargs or {}))
        results = res.results
        kernel.last_results = res
        _FIRST_CALL_DONE = True
    else:
        results = _run_cached(in_maps)
    out = np.empty((B, T, C), np.float32)
    for r in range(N_CORES):
        b, g = divmod(r, G)
        out[b, :, g * CG:(g + 1) * CG] = results[r]["out_t"].T
    return out


_FIRST_CALL_DONE = False


# revision 13
# speedup vs baseline: 1.4193x; 1.0715x over previous
"""Trainium2 Bass kernel for DecoderMultiHeadAttention (B=2, T=2048, C=768, H=12).

Sharding: 8 cores = 2 batches x 4 head-groups (3 heads each).
Per core: K,V projections for its head group from a host-pre-transposed
x^T (bf16), RoPE in deinterleaved pair layout (host permutes W_att
columns per head to [evens|odds]; W_proj rows permuted to match), causal
flash-style attention with transposed score layout in bf16, AllGather of
f16 attention outputs within each batch group of 4 cores, then a
column-sharded output projection reading the gathered f16 directly.

Note: the reference uses q = rope(v) (faithful source bug), so the
q-chunk of W_att (columns C..2C) is never used and is not computed.
"""

import sys

_REPO = "/opt/trn_rl_repo"
if _REPO not in sys.path:
    sys.path.insert(0, _REPO)

import numpy as np

import concourse.bass as bass
import concourse.mybir as mybir
import concourse.tile as tile
from concourse import bacc
from concourse.bass_utils import run_bass_kernel_spmd
from concourse.masks import make_identity

B, T, C, H = 2, 2048, 768, 12
D = C // H            # 64
N_CORES = 8
G = 4                 # head groups
HG = H // G           # 3 heads per group
CG = HG * D           # 192 output columns per group
NT = T // 128         # 16 t-chunks
NCC = C // 128        # 6 c-chunks
TQ = 512              # q block width
F32 = mybir.dt.float32
F32R = mybir.dt.float32r
BF16 = mybir.dt.bfloat16
F16 = mybir.dt.float16
EXP = mybir.ActivationFunctionType.Exp
SCALE = float(D) ** -0.5


def _body(nc, tc, xt, wkv, wp, bp, cos3, sin3, out_t, sim_variant=False, reps=1):
    with tc.tile_pool(name="const", bufs=1) as cp:
        ident = cp.tile([128, 128], F32)
        make_identity(nc, ident[:])
        identB = cp.tile([128, 128], BF16)
        nc.gpsimd.tensor_copy(identB[:], ident[:])
        # tri[p, f] = 1.0 if f >= p else 0.0  (keep tq >= tk in diagonal blocks)
        tri_f = cp.tile([128, 128], F32)
        nc.gpsimd.memset(tri_f[:], 1.0)
        nc.gpsimd.affine_select(
            out=tri_f[:], in_=tri_f[:], compare_op=mybir.AluOpType.is_ge,
            fill=0.0, base=0, pattern=[[1, 128]], channel_multiplier=-1)
        tri = cp.tile([128, 128], BF16)
        nc.gpsimd.tensor_copy(tri[:], tri_f[:])

        # x^T staged whole in SBUF (bf16, 24KB/partition), loaded in quarters;
        # quarter 0 issued first so the first KV matmul starts ASAP.
        xT_all = cp.tile([128, NCC * T], BF16)
        nc.sync.dma_start(
            xT_all[:].rearrange("p (n m) -> p n m", n=NCC)[:, :, 0:TQ],
            xt[:, 0:TQ].rearrange("(n p) m -> p n m", p=128))
        # weights (host layouts: wkv [128, NCC*2CG] bf16, wp [128, NCC*CG] f16)
        wkv_sb = cp.tile([128, NCC * 2 * CG], BF16)
        nc.scalar.dma_start(wkv_sb[:], wkv)
        for q in range(1, 4):
            nc.sync.dma_start(
                xT_all[:].rearrange("p (n m) -> p n m", n=NCC)
                [:, :, q * TQ:(q + 1) * TQ],
                xt[:, q * TQ:(q + 1) * TQ].rearrange("(n p) m -> p n m", p=128))
        # rope tables, deinterleaved pair layout: per chunk i, cols
        # [i*96 + h*32 + j] = cos/sin(ang[i*128+p, j])
        cos_sb = cp.tile([128, NT * 96], BF16)
        sin_sb = cp.tile([128, NT * 96], BF16)
        nc.scalar.dma_start(cos_sb[:], cos3)
        nc.scalar.dma_start(sin_sb[:], sin3)
        wp_sb = cp.tile([128, NCC * CG], F16)
        nc.scalar.dma_start(wp_sb[:], wp)
        bp_sb = cp.tile([1, CG], F16)
        nc.scalar.dma_start(bp_sb[:], bp)
        ones_f = cp.tile([1, TQ], F16)
        nc.gpsimd.memset(ones_f[:], 1.0)

        # persistent per-head [D, T] tensors (bf16): heads 0,1 packed
        kT01 = cp.tile([128, T], BF16)
        kT2 = cp.tile([64, T], BF16)
        qT01 = cp.tile([128, T], BF16)
        qT2 = cp.tile([64, T], BF16)
        # V in [T, D] layout with a ones column appended per head
        vaug = cp.tile([128, NT * (HG * 65)], BF16)
        ones48 = cp.tile([128, NT * HG], BF16)
        nc.gpsimd.memset(ones48[:], 1.0)
        nc.gpsimd.tensor_copy(
            vaug[:].rearrange("p (k c) -> p k c", c=65)[:, :, 64], ones48[:])
        # attention output, transposed [CG, T] f16, one tile pair per split
        SPLITS = [(0, 512), (512, 512), (1024, 512),
                  (1536, 256), (1792, 256)]
        oT01h = [cp.tile([128, bw], F16, name=f"oT01h{k}")
                 for k, (bs_, bw) in enumerate(SPLITS)]
        oT2h = [cp.tile([64, bw], F16, name=f"oT2h{k}")
                for k, (bs_, bw) in enumerate(SPLITS)]

        for _rep in range(reps):
            # ---- Stage 1: KV projection + RoPE + per-head transposes ----
            with tc.tile_pool(name="s1", bufs=3) as s1, \
                 tc.tile_pool(name="s2", bufs=2) as s2, \
                 tc.tile_pool(name="s3", bufs=1) as s3:
              with tc.tile_pool(name="s1ps", bufs=2, space="PSUM") as s1ps:
                for i in range(NT):
                    kv_ps = s1ps.tile([128, 2 * CG], F32, tag="kv", bufs=2)
                    for c in range(NCC):
                        nc.tensor.matmul(
                            kv_ps[:],
                            xT_all[:, c * T + i * 128:c * T + (i + 1) * 128],
                            wkv_sb[:, c * 2 * CG:(c + 1) * 2 * CG],
                            start=(c == 0), stop=(c == NCC - 1))
                    # K half staged to SBUF bf16 (Pool); V half into vaug (DVE)
                    ksb = s1.tile([128, CG], BF16, tag="ksb", bufs=2)
                    nc.scalar.copy(ksb[:], kv_ps[:, 0:CG])
                    vdst = vaug[:, i * 195:(i + 1) * 195] \
                        .rearrange("p (h c) -> p h c", h=HG)[:, :, 0:64]
                    nc.vector.tensor_copy(
                        vdst, kv_ps[:, CG:2 * CG]
                        .rearrange("p (h c) -> p h c", h=HG))

                    # RoPE in deinterleaved layout: [xr(32)|xi(32)] per head.
                    # or = xr*c - xi*s ; oi = xr*s + xi*c
                    kq = s1.tile([128, 2 * CG], BF16, tag="kq", bufs=2)
                    cS = cos_sb[:, i * 96:(i + 1) * 96] \
                        .rearrange("p (h d) -> p h d", h=HG)
                    sS = sin_sb[:, i * 96:(i + 1) * 96] \
                        .rearrange("p (h d) -> p h d", h=HG)
                    vsrc = vaug[:, i * 195:(i + 1) * 195] \
                        .rearrange("p (h c) -> p h c", h=HG)
                    for half, src in ((0, ksb[:].rearrange(
                            "p (h q d) -> p h q d", h=HG, q=2)),
                            (1, vsrc[:, :, 0:64].rearrange(
                                "p h (q d) -> p h q d", q=2))):
                        xr = src[:, :, 0, :]
                        xi = src[:, :, 1, :]
                        dst = kq[:, half * CG:(half + 1) * CG] \
                            .rearrange("p (h q d) -> p h q d", h=HG, q=2)
                        m1 = s1.tile([128, 96], BF16, tag=f"m1{half}", bufs=2)
                        m2 = s1.tile([128, 96], BF16, tag=f"m2{half}", bufs=2)
                        m3 = s1.tile([128, 96], BF16, tag=f"m3{half}", bufs=2)
                        m4 = s1.tile([128, 96], BF16, tag=f"m4{half}", bufs=2)
                        m1v = m1[:].rearrange("p (h d) -> p h d", h=HG)
                        m2v = m2[:].rearrange("p (h d) -> p h d", h=HG)
                        m3v = m3[:].rearrange("p (h d) -> p h d", h=HG)
                        m4v = m4[:].rearrange("p (h d) -> p h d", h=HG)
                        nc.vector.tensor_mul(m1v, xr, cS)
                        nc.gpsimd.tensor_mul(m2v, xi, sS)
                        nc.vector.tensor_sub(dst[:, :, 0, :], m1v, m2v)
                        nc.vector.tensor_mul(m3v, xr, sS)
                        nc.gpsimd.tensor_mul(m4v, xi, cS)
                        nc.vector.tensor_add(dst[:, :, 1, :], m3v, m4v)

                    # transpose roped K and Q into [D, T] per-head
                    # layouts; copies batched over chunk pairs (half the
                    # per-op ACT init overhead)
                    if i % 2 == 0:
                        tp = s1ps.tile([128, 1024], BF16, tag="tp", bufs=2)
                        tp_pair = tp
                    else:
                        tp = tp_pair
                    po = (i % 2) * 512
                    nc.tensor.transpose(tp[:, po:po + 128], kq[:, 0:128],
                                        identB[:])
                    nc.tensor.transpose(tp[0:64, po + 128:po + 256],
                                        kq[:, 128:192], identB[:])
                    nc.tensor.transpose(tp[:, po + 256:po + 384],
                                        kq[:, 192:320], identB[:])
                    nc.tensor.transpose(tp[0:64, po + 384:po + 512],
                                        kq[:, 320:384], identB[:])
                    if i % 2 == 1:
                        ts2 = slice((i - 1) * 128, (i + 1) * 128)
                        tpv = tp[:].rearrange("p (two x) -> p two x", two=2)
                        tpv64 = tp[0:64].rearrange(
                            "p (two x) -> p two x", two=2)
                        nc.scalar.copy(
                            kT01[:, ts2].rearrange(
                                "p (two m) -> p two m", two=2),
                            tpv[:, :, 0:128])
                        nc.scalar.copy(
                            kT2[:, ts2].rearrange(
                                "p (two m) -> p two m", two=2),
                            tpv64[:, :, 128:256])
                        nc.scalar.copy(
                            qT01[:, ts2].rearrange(
                                "p (two m) -> p two m", two=2),
                            tpv[:, :, 256:384])
                        nc.scalar.copy(
                            qT2[:, ts2].rearrange(
                                "p (two m) -> p two m", two=2),
                            tpv64[:, :, 384:512])

              with tc.tile_pool(name="s23ps", bufs=2, space="PSUM") as s2ps:
                  s3ps = s2ps
                  dp = tc.alloc_tile_pool(name="dram", bufs=1, space="DRAM")
                  # staged stage 3: per-part AllGather fires as soon as its
                  # splits' oT are done; its projection is emitted one
                  # attention-split later so PE never stalls on the collective
                  PARTS = [((0, 1), (0, 1024)), ((2,), (1024, 1536)),
                           ((3,), (1536, 1792)), ((4,), (1792, 2048))]

                  def attn_split(b):
                      bs, bw = SPLITS[b]
                      for h in range(HG):
                          kT = (kT01[0:64], kT01[64:128], kT2[0:64])[h]
                          qT = (qT01[0:64], qT01[64:128], qT2[0:64])[h]
                          oT = (oT01h[b][0:64], oT01h[b][64:128],
                                oT2h[b][0:64])[h]
                          nblk = (bs + bw) // 128
                          # rows 0:64 = AV, row 64 = Z
                          o_ps = s2ps.tile([128, TQ], F32, tag="o", bufs=2)

                          packs, cur, w = [], [], 0
                          for t in range(nblk):
                              diag = t * 128 >= bs
                              col0 = t * 128 - bs if diag else 0
                              ncols = bw - col0
                              if w + ncols > 2 * TQ:
                                  packs.append(cur)
                                  cur, w = [], 0
                              cur.append((t, col0, ncols, w, diag))
                              w += ncols
                          if cur:
                              packs.append(cur)
                          for pk in packs:
                              pw = sum(c[2] for c in pk)
                              s_ps = s2ps.tile([128, 2 * TQ], F32, tag="s",
                                               bufs=2)
                              wei = s2.tile([128, 2 * TQ], BF16, tag="wei",
                                            bufs=4)
                              for t, col0, ncols, off, diag in pk:
                                  nc.tensor.matmul(
                                      s_ps[:, off:off + ncols],
                                      kT[:, t * 128:(t + 1) * 128],
                                      qT[:, bs + col0:bs + bw],
                                      start=True, stop=True)
                              nc.scalar.activation(wei[:, 0:pw], s_ps[:, 0:pw],
                                                   EXP, scale=SCALE)
                              for t, col0, ncols, off, diag in pk:
                                  if diag:
                                      nc.vector.tensor_mul(
                                          wei[:, off:off + 128],
                                          wei[:, off:off + 128], tri[:])
                                  va = t * 195 + h * 65
                                  nc.tensor.matmul(
                                      o_ps[0:65, col0:bw],
                                      vaug[:, va:va + 65],
                                      wei[:, off:off + ncols],
                                      start=(t == 0), stop=(t == nblk - 1))
                          recip = s2.tile([1, TQ], F32, tag="recip", bufs=2)
                          nc.vector.reciprocal(recip[:, 0:bw],
                                               o_ps[64:65, 0:bw])
                          rb = s2.tile([64, TQ], F32, tag="rb", bufs=2)
                          nc.gpsimd.partition_broadcast(rb[:, 0:bw],
                                                        recip[:, 0:bw])
                          nc.vector.tensor_mul(oT[:], o_ps[0:64, 0:bw],
                                               rb[:, 0:bw])

                  ag_outs = {}

                  def coll_part(splits, c0, c1):
                      w = c1 - c0
                      ag_in = dp.tile([CG, w], F16, name=f"agi{c0}")
                      ag_out = dp.tile([G * CG, w], F16, name=f"ago{c0}")
                      ag_outs[c0] = ag_out
                      for q in splits:
                          qbs, qbw = SPLITS[q]
                          nc.sync.dma_start(
                              ag_in[0:128, qbs - c0:qbs - c0 + qbw],
                              oT01h[q][:])
                          nc.sync.dma_start(
                              ag_in[128:CG, qbs - c0:qbs - c0 + qbw],
                              oT2h[q][:])
                      if sim_variant:
                          nc.sync.dma_start(
                              ag_out[:].rearrange("(g r) w -> g r w", g=G),
                              ag_in[:].rearrange("(g r) w -> g r w", g=1)
                              .broadcast_to([G, CG, w]))
                      else:
                          nc.gpsimd.collective_compute(
                              "AllGather", mybir.AluOpType.bypass,
                              replica_groups=[[0, 1, 2, 3], [4, 5, 6, 7]],
                              ins=[ag_in[:].opt()], outs=[ag_out[:].opt()])

                  def proj_part(c0, c1):
                      ag_out = ag_outs[c0]
                      pos = c0
                      while pos < c1:
                          w = min(TQ, c1 - pos)
                          a_bf = s3.tile([128, NCC * TQ], F16, tag="abf",
                                         bufs=2)
                          nc.sync.dma_start(
                              a_bf[:].rearrange("p (n m) -> p n m", n=NCC)
                              [:, :, 0:w],
                              ag_out[:, pos - c0:pos - c0 + w]
                              .rearrange("(n p) m -> p n m", p=128))
                          p01 = s3ps.tile([128, 2 * TQ], F32, tag="p01",
                                          bufs=1)
                          p0 = p01[:, 0:w]
                          p1 = p01[0:64, TQ:TQ + w]
                          for c in range(NCC):
                              a_sb = a_bf[:, c * TQ:c * TQ + w]
                              nc.tensor.matmul(
                                  p0, wp_sb[:, c * CG:c * CG + 128],
                                  a_sb, start=(c == 0), stop=False)
                              nc.tensor.matmul(
                                  p1, wp_sb[:, c * CG + 128:(c + 1) * CG],
                                  a_sb, start=(c == 0), stop=False)
                          nc.tensor.matmul(p0, bp_sb[:, 0:128],
                                           ones_f[:, 0:w],
                                           start=False, stop=True)
                          nc.tensor.matmul(p1, bp_sb[:, 128:CG],
                                           ones_f[:, 0:w],
                                           start=False, stop=True)
                          o_all0 = s3.tile([128, TQ], F32, tag="oall0",
                                           bufs=2)
                          o_all1 = s3.tile([64, TQ], F32, tag="oall1",
                                           bufs=2)
                          nc.vector.tensor_copy(o_all0[:, 0:w], p0)
                          nc.scalar.copy(o_all1[:, 0:w], p1)
                          qs = slice(pos, pos + w)
                          nc.scalar.dma_start(out_t[0:128, qs],
                                              o_all0[:, 0:w])
                          nc.scalar.dma_start(out_t[128:CG, qs],
                                              o_all1[:, 0:w])
                          pos += w

                  attn_split(0)
                  attn_split(1)
                  coll_part(*PARTS[0][0:1], *PARTS[0][1])
                  attn_split(2)
                  proj_part(*PARTS[0][1])
                  coll_part(*PARTS[1][0:1], *PARTS[1][1])
                  attn_split(3)
                  proj_part(*PARTS[1][1])
                  coll_part(*PARTS[2][0:1], *PARTS[2][1])
                  attn_split(4)
                  proj_part(*PARTS[2][1])
                  coll_part(*PARTS[3][0:1], *PARTS[3][1])
                  proj_part(*PARTS[3][1])


def _build(sim_variant=False, reps=1):
    nc = bacc.Bacc("TRN2", target_bir_lowering=False, debug=False,
                   num_devices=1 if sim_variant else N_CORES,
                   enable_asserts=False)
    xt = nc.dram_tensor("xt", [C, T], BF16, kind="ExternalInput").ap()
    wkv = nc.dram_tensor("wkv", [128, NCC * 2 * CG], BF16,
                         kind="ExternalInput").ap()
    wp = nc.dram_tensor("wp", [128, NCC * CG], F16, kind="ExternalInput").ap()
    bp = nc.dram_tensor("bp", [1, CG], F16, kind="ExternalInput").ap()
    cos3 = nc.dram_tensor("cos3", [128, NT * 96], BF16,
                          kind="ExternalInput").ap()
    sin3 = nc.dram_tensor("sin3", [128, NT * 96], BF16,
                          kind="ExternalInput").ap()
    out_t = nc.dram_tensor("out_t", [CG, T], F32, kind="ExternalOutput").ap()
    with tile.TileContext(nc) as tc:
        _body(nc, tc, xt, wkv, wp, bp, cos3, sin3, out_t, sim_variant, reps)
    nc.compile()
    return nc


_NC = None


def _get_nc():
    global _NC
    if _NC is None:
        _NC = _build()
    return _NC


_EXEC = None


def _get_exec():
    global _EXEC
    if _EXEC is None:
        _EXEC = _make_exec(_get_nc())
    return _EXEC


def _make_exec(nc):
    """Reusable jitted SPMD executable (mirrors bass2jax.run_bass_via_pjrt's
    multi-core path)."""
    import jax
    from jax.experimental.shard_map import shard_map
    from jax.sharding import Mesh, PartitionSpec
    from concourse import bass2jax, mybir as _mybir

    bass2jax.install_neuronx_cc_hook()
    in_names, out_names, out_avals, zero_outs = [], [], [], []
    assert nc.dbg_addr is None
    pname = nc.partition_id_tensor.name if nc.partition_id_tensor else None
    for alloc in nc.m.functions[0].allocations:
        if not isinstance(alloc, _mybir.MemoryLocationSet):
            continue
        name = alloc.memorylocations[0].name
        if alloc.kind == "ExternalInput":
            if name != pname:
                in_names.append(name)
        elif alloc.kind == "ExternalOutput":
            out_names.append(name)
            shape = tuple(alloc.tensor_shape)
            dtype = _mybir.dt.np(alloc.dtype)
            out_avals.append(jax.core.ShapedArray(shape, dtype))
            zero_outs.append(np.zeros(shape, dtype))
    n_params = len(in_names)
    all_names = in_names + out_names
    if pname is not None:
        all_names = all_names + [pname]

    def _fn(*args):
        operands = list(args)
        if pname is not None:
            operands.append(bass2jax.partition_id_tensor())
        outs = bass2jax._bass_exec_p.bind(
            *operands,
            out_avals=tuple(out_avals),
            in_names=tuple(all_names),
            out_names=tuple(out_names),
            lowering_input_output_aliases=(),
            sim_require_finite=True,
            sim_require_nnan=True,
            nc=nc,
        )
        return tuple(outs)

    devices = jax.devices()[:N_CORES]
    mesh = Mesh(np.asarray(devices), ("core",))
    nin = n_params + len(out_names)
    donate = tuple(range(n_params, n_params + len(out_names)))
    sharded = jax.jit(
        shard_map(_fn, mesh=mesh,
                  in_specs=(PartitionSpec("core"),) * nin,
                  out_specs=(PartitionSpec("core"),) * len(out_names),
                  check_rep=False),
        donate_argnums=donate, keep_unused=True)

    def _zero_cat():
        return [np.zeros((N_CORES * z.shape[0], *z.shape[1:]), z.dtype)
                for z in zero_outs]

    return (sharded, in_names, out_names, out_avals, _zero_cat)


def _run_cached(in_maps):
    sharded, in_names, out_names, out_avals, zero_cat = _get_exec()
    concat_in = [np.concatenate([np.asarray(in_maps[c][n])
                                 for c in range(N_CORES)], axis=0)
                 for n in in_names]
    out_arrs = sharded(*concat_in, *zero_cat())
    return [
        {name: np.asarray(out_arrs[i]).reshape(N_CORES, *out_avals[i].shape)[c]
         for i, name in enumerate(out_names)}
        for c in range(N_CORES)
    ]


_PERM = np.concatenate([np.arange(0, 64, 2), np.arange(1, 64, 2)])


def _prep_rope(r, np_bf16):
    # [T, 32] -> [128, NT*96]: chunk i, cols i*96 + h*32 + j = r[i*128+p, j]
    rr = r.reshape(NT, 128, 32).transpose(1, 0, 2)           # [128, NT, 32]
    rr = np.broadcast_to(rr[:, :, None, :], (128, NT, HG, 32))
    return np.ascontiguousarray(rr.reshape(128, NT * 96)).astype(np_bf16)


def _cm(w, cols):
    # [768, cols] -> c-chunk-major [128, NCC*cols]
    return np.ascontiguousarray(
        w.reshape(NCC, 128, -1).transpose(1, 0, 2).reshape(128, -1))


def _shard_inputs(x, rope_cos, rope_sin, W_att, W_proj, b_proj):
    np_bf16 = mybir.dt.np(BF16)
    x = np.asarray(x, np.float32)
    W_att = np.asarray(W_att, np.float32)
    W_proj = np.asarray(W_proj, np.float32)
    b_proj = np.asarray(b_proj, np.float32)
    cos3 = _prep_rope(np.asarray(rope_cos, np.float32), np_bf16)
    sin3 = _prep_rope(np.asarray(rope_sin, np.float32), np_bf16)
    # W_proj rows permuted to match deinterleaved V columns (all heads)
    rowp = np.concatenate([h * 64 + _PERM for h in range(H)])
    wp_perm = W_proj[rowp]
    xts = [np.ascontiguousarray(x[b].T).astype(np_bf16) for b in range(B)]
    in_maps = []
    for r in range(N_CORES):
        b, g = divmod(r, G)
        heads = [3 * g + h for h in range(HG)]
        kcols = np.concatenate([h * 64 + _PERM for h in heads])
        wkv = np.concatenate(
            [W_att[:, kcols], W_att[:, 2 * C + kcols]], axis=1)  # [768, 384]
        in_maps.append({
            "xt": xts[b],
            "wkv": _cm(wkv, 2 * CG).astype(np_bf16),
            "wp": _cm(wp_perm[:, g * CG:(g + 1) * CG], CG).astype(np.float16),
            "bp": np.ascontiguousarray(
                b_proj[g * CG:(g + 1) * CG][None, :]).astype(np.float16),
            "cos3": cos3,
            "sin3": sin3,
        })
    return in_maps


def kernel(x, rope_cos, rope_sin, W_att, W_proj, b_proj, _run_kwargs=None):
    nc = _get_nc()
    in_maps = _shard_inputs(x, rope_cos, rope_sin, W_att, W_proj, b_proj)
    global _FIRST_CALL_DONE, _last_in_maps
    _last_in_maps = in_maps
    if not _FIRST_CALL_DONE:
        res = run_bass_kernel_spmd(nc, in_maps, core_ids=list(range(N_CORES)),
                                   **(_run_kwargs or {}))
        results = res.results
        kernel.last_results = res
        _FIRST_CALL_DONE = True
    else:
        results = _run_cached(in_maps)
    out = np.empty((B, T, C), np.float32)
    for r in range(N_CORES):
        b, g = divmod(r, G)
        out[b, :, g * CG:(g + 1) * CG] = results[r]["out_t"].T
    return out


_FIRST_CALL_DONE = False
